# revision 52
# baseline (speedup 1.0000x reference)
"""Causal self-attention (B=2, T=2048, D=2048, H=16, hd=128, RoPE on masked
heads) as a Bass/Tile kernel on 8 Trainium2 NeuronCores.

Sharding: core c handles batch b=c//4 and heads 4*(c%4)..4*(c%4)+3 (data
parallel on B x tensor parallel on H).  Each core computes a partial output
projection y_b = O_local @ Wout_local^T; the host sums the 4 partials per
batch.

Numerics/performance strategy:
- QKV projection runs as fp8(e4m3) DoubleRow matmuls with 3-term residual
  compensation: x*W ~ x8*W8 + xr8*W8 + x8*Wr8, where xr8/Wr8 are e4m3
  quantizations of the quantization residuals (host-prepared).  Each
  DoubleRow instruction contracts two 128-row K-slabs at half cost, so the
  projection runs at 1.5x the bf16 matmul rate with ~1e-3 relative error.
  The 3 terms are packed into 24 DoubleRow instructions per output tile via
  a chain pairing that needs no operand duplication (see _emit_3term).
- Attention (scores, softmax, PV, denominator) runs in bf16: S^T = K Q^T in
  transposed score space so softmax normalization is a per-free-element
  multiply; denominator via a constant-value stationary matmul whose value
  folds the fp8 descale so normalized outputs land in e4m3 range.  Score
  tiles are computed in PAIRS sharing one PSUM bank (only the first matmul
  carries start; region-granular zeroing verified on hw) so one exp covers
  two k-blocks, keeping the Activation engine off the critical path.
- The output projection also runs as 3-term fp8 DoubleRow: the normalize
  step emits an e4m3 hi part plus an exact residual lo part (one extra DVE
  subtract per tile), contracted against host-prepared Wout hi/lo slabs.
- q, k, v stay resident in SBUF between phases (bf16) - no DRAM scratch;
  y is written as bf16 partials and summed in f32 on the host.
- RoPE tables are a single per-core C/S pair (identity for NoPE cores);
  roped = C*q + S*(J q) with J applied as a PE matmul; head-0 rope is
  folded into phase 1 and phase-3 tiles are interleaved into the last
  head's attention so the PE pipeline never drains between phases.
"""

import sys

sys.path.insert(0, "/opt/trn_rl_repo")

import numpy as np

import concourse.bass as bass
import concourse.mybir as mybir
import concourse.tile as tile
from concourse.bass_utils import run_bass_kernel_spmd

F32 = mybir.dt.float32
F8 = mybir.dt.float8e4
BF16 = mybir.dt.bfloat16
DR = mybir.MatmulPerfMode.DoubleRow

B = 2
T = 2048
D = 2048
H = 16
HD = 128
N_CORES = 8
HPC = 4           # heads per core
CORES_PER_B = 4
P = 128
TB = 512          # t-block width (phase 1 / rope)
NTB = T // TB     # 4
TQ = 256          # attention q-tile width (phase 2)
NTQ = T // TQ     # 8
BANDS = TQ // P   # 2
KO = D // P       # 16 contraction K-blocks of 128
NQK = 2 * HPC     # 8 q+k dout blocks of 128
SX = 16.0         # fp8 scale for x
SW = 1024.0       # fp8 scale for Wqkv
SWO = 1024.0      # fp8 scale for Wout
SO = 32.0         # fp8 scale carried by the normalized attention output
SIGMA = SX * SW   # scale carried by q,k,v in SBUF
SCALE_EFF = (1.0 / float(np.sqrt(HD))) / (SIGMA * SIGMA)


# ---------------------------------------------------------------------------
# Walrus on this toolchain rejects instructions carrying more than one sync
# wait command; Tile can emit several (e.g. the kernel-tail drain).  Hoist
# the excess onto injected same-engine NoOps — semantically identical.
def _fix_waits(nc, cap=1):
    ctr = 0
    for f in nc.m.functions:
        for bb in f.blocks:
            insts = bb.instructions
            i = 0
            while i < len(insts):
                inst = insts[i]
                si = inst.sync_info
                if si is not None and si.on_wait and len(si.on_wait) > cap:
                    waits = list(si.on_wait)
                    keep, excess = waits[:cap], waits[cap:]
                    nops = []
                    for j in range(0, len(excess), cap):
                        ctr += 1
                        nops.append(
                            mybir.InstNoOp(
                                name=f"I-waitfix-{ctr}",
                                engine=inst.engine,
                                sync_info=mybir.SyncInfo(
                                    on_wait=excess[j : j + cap], on_update=[]
                                ),
                            )
                        )
                    inst.sync_info = mybir.SyncInfo(
                        on_wait=keep, on_update=list(si.on_update or [])
                    )
                    insts[i:i] = nops
                    i += len(nops)
                i += 1
    return ctr


def _emit_3term(nc, ps, w_sb, wr_sb, xs_t, msl, tsl, w_of_pair, x_of_pair):
    """Emit the 24 DoubleRow matmuls of one 3-term-compensated K=2048
    contraction into PSUM tile `ps`.

    xs_t holds 32 K-slabs (2i = x8_i, 2i+1 = xr8_i); w_sb/wr_sb hold 16
    slabs each (W8_i / Wr8_i).  Chain pairing covers x8_i*W8_i, xr8_i*W8_i
    (A instructions) and x8_i*Wr8_i (B instructions) with constant-stride
    slab pairs only.  `w_of_pair(w_tile, s0, s1, msl)` / `x_of_pair(xs, s0,
    s1, tsl)` build the [128, 2, *] APs (orientation differs between the
    q/k and v sweeps).
    """
    seq = _3term_seq(w_sb, wr_sb, xs_t, msl, tsl, w_of_pair, x_of_pair)
    n = len(seq)
    for i, (w_ap, x_ap) in enumerate(seq):
        nc.tensor.matmul(ps[:], w_ap, x_ap, start=(i == 0), stop=(i == n - 1), perf_mode=DR)


def _3term_seq(w_sb, wr_sb, xs_t, msl, tsl, w_of_pair, x_of_pair):
    seq = []
    # A_1..A_15: x slabs (2j-1, 2j), w slabs (j-1, j)
    for j in range(1, KO):
        seq.append((w_of_pair(w_sb, j - 1, j, msl), x_of_pair(xs_t, 2 * j - 1, 2 * j, tsl)))
    # B_0..B_7: x slabs (4m, 4m+2), wr slabs (2m, 2m+1)
    for m in range(KO // 2):
        seq.append((w_of_pair(wr_sb, 2 * m, 2 * m + 1, msl), x_of_pair(xs_t, 4 * m, 4 * m + 2, tsl)))
    # A_0: x slabs (0, 31), w slabs (0, 15)
    seq.append((w_of_pair(w_sb, 0, KO - 1, msl), x_of_pair(xs_t, 0, 2 * KO - 1, tsl)))
    return seq


def _slab_pair(t, s0, s1, csl):
    """AP [128, 2, cols] selecting slabs s0 < s1 of a [P, nslab, C] tile."""
    if csl is None:
        return t[:, s0 : s1 + 1 : (s1 - s0), :] if s1 - s0 > 1 else t[:, s0 : s1 + 1, :]
    step = s1 - s0
    if step > 1:
        return t[:, s0 : s1 + 1 : step, csl]
    return t[:, s0 : s1 + 1, csl]


def _rope_block(nc, psum_pool, tmp_pool, qk_sb, cs_sb, jT_sb, h, qr, kr, rb, tag="psj", psj_bufs=2):
    """RoPE for one 512-wide t-block of head h: roped = C*q + S*(J q).
    qr/kr are per-t-block tile lists so consumers only depend on their own
    block's blend, not the whole head."""
    sl = slice(rb * TB, (rb + 1) * TB)
    for si, (src_t, dst) in enumerate(((qk_sb[h], qr[rb]), (qk_sb[HPC + h], kr[rb]))):
        psj = psum_pool.tile([P, TB], F32, tag=tag, name=f"psj{h}_{rb}_{si}", bufs=psj_bufs)
        nc.tensor.matmul(psj[:], jT_sb[:], src_t[:, sl], start=True, stop=True)
        tmp = tmp_pool.tile([P, TB], BF16, tag="ropetmp", name=f"rtmp{h}_{rb}_{si}")
        nc.vector.tensor_tensor(tmp[:], psj[:], cs_sb[:, 1, sl], mybir.AluOpType.mult)
        nc.vector.tensor_tensor(dst[:], src_t[:, sl], cs_sb[:, 0, sl], mybir.AluOpType.mult)
        nc.vector.tensor_tensor(dst[:], dst[:], tmp[:], mybir.AluOpType.add)


def _phase1(nc, tc, xs, wqks, wqkrs, wvs, wvrs, qk_sb, v_sb, rope0, const_dmas):
    with (
        tc.tile_pool(name="p1w", bufs=1) as p1w,
        tc.tile_pool(name="p1x", bufs=2) as p1x,
        tc.tile_pool(name="p1t", bufs=2) as p1t,
        tc.tile_pool(name="p1p", bufs=3, space="PSUM") as p1p,
        tc.tile_pool(name="p1pj", bufs=1, space="PSUM") as p1pj,
    ):
        wqk_t = p1w.tile([P, NQK, KO, P], F8, name="wqks")
        wqkr_t = p1w.tile([P, NQK, KO, P], F8, name="wqkrs")
        wv_t = p1w.tile([P, KO, HPC * HD], F8, name="wvs")
        wvr_t = p1w.tile([P, KO, HPC * HD], F8, name="wvrs")

        # q/k sweep: stationary = weight slab pair, moving = x slab pair
        def w_qk(t, s0, s1, msl):
            return _slab_pair(t, s0, s1, msl)

        def x_qk(t, s0, s1, _):
            return _slab_pair(t, s0, s1, None)

        first = True
        for tb in range(NTB):
            tsl = slice(tb * TB, (tb + 1) * TB)
            xs_t = p1x.tile([P, 2 * KO, TB], F8, tag="xs", name=f"xs{tb}")
            if first:
                # fine-grained first loads so the first m-block's A chain can
                # start after ~1MB instead of ~4MB of DMA
                nc.sync.dma_start(xs_t[:, 0 : KO // 4, :], xs[:, 0 : KO // 4, tsl])
                nc.sync.dma_start(wqk_t[:, 0], wqks[:, 0])
                nc.sync.dma_start(xs_t[:, KO // 4 : KO // 2, :], xs[:, KO // 4 : KO // 2, tsl])
                nc.sync.dma_start(wqk_t[:, 1], wqks[:, 1])
                nc.sync.dma_start(xs_t[:, KO // 2 : KO, :], xs[:, KO // 2 : KO, tsl])
                nc.sync.dma_start(xs_t[:, KO : 2 * KO, :], xs[:, KO : 2 * KO, tsl])
                nc.sync.dma_start(wqkr_t[:, 0:2], wqkrs[:, 0:2])
                nc.sync.dma_start(wqk_t[:, 2:4], wqks[:, 2:4])
                nc.sync.dma_start(wqkr_t[:, 2:4], wqkrs[:, 2:4])
                nc.sync.dma_start(wqk_t[:, 4:8], wqks[:, 4:8])
                nc.sync.dma_start(wqkr_t[:, 4:8], wqkrs[:, 4:8])
                nc.sync.dma_start(wv_t[:], wvs[:])
                nc.sync.dma_start(wvr_t[:], wvrs[:])
                # const loads ride behind the critical phase-1 loads
                for dma in const_dmas:
                    dma()
                first = False
            else:
                nc.sync.dma_start(xs_t[:, 0:KO, :], xs[:, 0:KO, tsl])
                nc.sync.dma_start(xs_t[:, KO : 2 * KO, :], xs[:, KO : 2 * KO, tsl])

            def w_qk_m_of(m):
                def w_qk_m(t, s0, s1, _msl, _m=m):
                    step = s1 - s0
                    if step > 1:
                        return t[:, _m, s0 : s1 + 1 : step, :]
                    return t[:, _m, s0 : s1 + 1, :]
                return w_qk_m

            for m in range(NQK):
                ps = p1p.tile([P, TB], F32, tag="ps1", name=f"psqk{tb}_{m}")
                _emit_3term(nc, ps, wqk_t, wqkr_t, xs_t, None, None, w_qk_m_of(m), x_qk)
                cp = (nc.vector.tensor_copy, nc.scalar.copy)[m % 2]
                cp(qk_sb[m][:, tsl], ps[:])
            qr0, kr0, cs_sb, jT_sb = rope0
            _rope_block(nc, p1pj, p1t, qk_sb, cs_sb, jT_sb, 0, qr0, kr0, tb,
                        tag="psj1", psj_bufs=1)
            for t4 in range(4):
                t4sl = slice(t4 * P, (t4 + 1) * P)
                ps = p1p.tile([P, HPC * HD], F32, tag="ps1", name=f"psv{tb}_{t4}")
                # v: out[t, hd] — stationary x slabs sliced to t4, moving wv
                seq = []
                for j in range(1, KO):
                    seq.append((_slab_pair(xs_t, 2 * j - 1, 2 * j, t4sl), _slab_pair(wv_t, j - 1, j, None)))
                for m2 in range(KO // 2):
                    seq.append((_slab_pair(xs_t, 4 * m2, 4 * m2 + 2, t4sl), _slab_pair(wvr_t, 2 * m2, 2 * m2 + 1, None)))
                seq.append((_slab_pair(xs_t, 0, 2 * KO - 1, t4sl), _slab_pair(wv_t, 0, KO - 1, None)))
                for i, (x_ap, w_ap) in enumerate(seq):
                    nc.tensor.matmul(ps[:], x_ap, w_ap, start=(i == 0), stop=(i == len(seq) - 1), perf_mode=DR)
                cp = (nc.vector.tensor_copy, nc.scalar.copy)[t4 % 2]
                cp(v_sb[tb * 4 + t4][:], ps[:])


def _phase2(nc, tc, outS, qk_sb, v_sb, jT_sb, mask_sb, ones_sb, cs_sb, r0, emit_p3, aux_pool, p2ps):
    with (
        tc.tile_pool(name="p2r", bufs=2) as p2r,
        tc.tile_pool(name="p2pt", bufs=10) as p2pt,
        tc.tile_pool(name="p2rec", bufs=4) as p2rec,
        tc.tile_pool(name="p2po", bufs=2, space="PSUM") as p2po,
    ):
        def alloc_roped(h):
            qr = [p2r.tile([P, TB], BF16, tag=f"qr{rb}", name=f"qr{h}_{rb}") for rb in range(NTB)]
            kr = [p2r.tile([P, TB], BF16, tag=f"kr{rb}", name=f"kr{h}_{rb}") for rb in range(NTB)]
            return qr, kr

        def attn_tq(h, tq, qr, kr, pending):
            """One q-tile of attention, software-pipelined over PAIRS of
            128-wide k-blocks: the two STs of a pair land in two PSUM banks
            of one tile so a single exp (and, on the diagonal, a single mask
            multiply) covers both.  PV/ones matmuls trail via `pending`."""
            sl = slice(tq * TQ, (tq + 1) * TQ)
            nk = (tq + 1) * BANDS
            # ps_o (PV) and ps_d (denominator) share one 2KB bank: the first
            # PV's start zeroes the whole region, so the denominator chain
            # never carries start (verified region-zero semantics on hw).
            ps_od = p2po.tile([P, 2, TQ], F32, tag="po", name=f"po{h}{tq}")
            ps_o = ps_od[:, 0, :]
            ps_d = ps_od[:, 1, :]

            def issue_pair(kp):
                # both STs of a pair share one 2KB bank: the first carries
                # start (zeroing the region), the second relies on the
                # region-granular pending-zero (verified on hw)
                ps_st = p2ps.tile([P, 2, TQ], F32, tag="st", name=f"st{h}{tq}{kp}")
                qr_t = qr[tq * TQ // TB]
                qsl = slice((tq * TQ) % TB, (tq * TQ) % TB + TQ)
                for j in range(2):
                    kb = 2 * kp + j
                    kr_t = kr[kb * P // TB]
                    ksl = slice((kb * P) % TB, (kb * P) % TB + P)
                    nc.tensor.matmul(
                        ps_st[:, j, :], kr_t[:, ksl], qr_t[:, qsl],
                        start=(j == 0), stop=(j == 1), skip_group_check=True,
                    )
                pt = p2pt.tile([P, 2, TQ], BF16, tag="pt", name=f"pt{h}{tq}{kp}")
                nc.scalar.activation(
                    pt[:], ps_st[:], mybir.ActivationFunctionType.Exp, scale=SCALE_EFF
                )
                if kp == tq:  # diagonal pair: mask both bands at once
                    nc.vector.tensor_tensor(pt[:], pt[:], mask_sb[:], mybir.AluOpType.mult)
                return pt

            def make_pv(kp, pt):
                def pv():
                    for j in range(2):
                        kb = 2 * kp + j
                        nc.tensor.matmul(
                            ps_o, v_sb[kb][:, h * HD : (h + 1) * HD], pt[:, j, :],
                            start=(kb == 0), stop=False, skip_group_check=True,
                        )
                        nc.tensor.matmul(
                            ps_d, ones_sb[:], pt[:, j, :], start=False,
                            stop=(kb == nk - 1), skip_group_check=True,
                        )
                    if 2 * kp + 1 == nk - 1:
                        rec = p2rec.tile([P, TQ], F32, tag="rec", name=f"rec{h}{tq}")
                        nc.vector.reciprocal(rec[:], ps_d)
                        ob = p2rec.tile([P, TQ], BF16, tag="ob", name=f"ob{h}{tq}")
                        nc.vector.tensor_tensor(ob[:], ps_o, rec[:], mybir.AluOpType.mult)
                        hi = outS[tq][:, 2 * h, :]
                        nc.vector.tensor_copy(hi, ob[:])
                        nc.vector.scalar_tensor_tensor(
                            outS[tq][:, 2 * h + 1, :], ob[:], 1.0, hi,
                            mybir.AluOpType.mult, mybir.AluOpType.subtract,
                        )
                return pv

            for kp in range(nk // 2):
                pt = issue_pair(kp)
                if len(pending) >= 3:
                    pending.pop(0)()
                pending.append(make_pv(kp, pt))

        # rope for head h+1 is interleaved into head h's attention (one
        # 512-wide t-block per pair of q-tiles); head 0 was roped inside
        # phase 1.  During the last head, phase-3 tiles are emitted one
        # q-tile behind so output projection overlaps the attention tail.
        roped = [r0]
        pending = []
        for h in range(HPC):
            if h + 1 < HPC:
                roped.append(alloc_roped(h + 1))
            qr, kr = roped[h]
            for tq in range(NTQ):
                attn_tq(h, tq, qr, kr, pending)
                if h + 1 < HPC:
                    if tq % 2 == 0:
                        _rope_block(nc, aux_pool, p2pt, qk_sb, cs_sb, jT_sb,
                                    h + 1, roped[h + 1][0], roped[h + 1][1], tq // 2,
                                    tag="ps3", psj_bufs=3)
                elif tq >= 2:
                    # two q-tiles behind: head-3's normalization for tq-2 is
                    # guaranteed emitted (pending is only 3 pairs deep)
                    emit_p3(tq - 2)
            if h == HPC - 1:
                while pending:
                    pending.pop(0)()
        emit_p3(NTQ - 2)
        emit_p3(NTQ - 1)


def _make_p3(nc, p3s, p3p, outS, wos_sb, wors_sb, y):
    ydescale = 1.0 / (SO * SWO)

    def emit_p3(tq):
        for tt in range(tq * BANDS, (tq + 1) * BANDS):
            off = (tt - tq * BANDS) * P
            osl = slice(off, off + P)
            ysb = p3s.tile([P, D], BF16, tag="ysb", name=f"ysb{tt}")
            last = tq == NTQ - 1
            for dd in range(D // TB):
                dsl = slice(dd * TB, (dd + 1) * TB)
                ps = p3p.tile([P, TB], F32, tag="ps3", name=f"ps3{tt}{dd}")
                seq = []
                for j in range(1, HPC):
                    seq.append((outS[tq][:, 2 * j - 1 : 2 * j + 1, osl], wos_sb[:, j - 1 : j + 1, dsl]))
                for m in range(HPC // 2):
                    seq.append((outS[tq][:, 4 * m : 4 * m + 3 : 2, osl], wors_sb[:, 2 * m : 2 * m + 2, dsl]))
                seq.append((outS[tq][:, 0 : 2 * HPC : 2 * HPC - 1, osl], wos_sb[:, 0 : HPC : HPC - 1, dsl]))
                for i, (o_ap, w_ap) in enumerate(seq):
                    nc.tensor.matmul(ps[:], o_ap, w_ap, start=(i == 0), stop=(i == len(seq) - 1), perf_mode=DR)
                if dd % 2 == 0:
                    nc.vector.tensor_scalar_mul(ysb[:, dsl], ps[:], ydescale)
                else:
                    nc.scalar.mul(ysb[:, dsl], ps[:], ydescale)
                if last:
                    # small per-dd stores shrink the end-of-kernel DMA tail
                    nc.sync.dma_start(y[tt * P : (tt + 1) * P, dsl], ysb[:, dsl])
            if not last:
                nc.sync.dma_start(y[tt * P : (tt + 1) * P, :], ysb[:])
    return emit_p3


def _build_program():
    nc = bass.Bass()

    xs = nc.dram_tensor("xs", (P, 2 * KO, T), F8, kind="ExternalInput")
    wqks = nc.dram_tensor("wqks", (P, NQK, KO, P), F8, kind="ExternalInput")
    wqkrs = nc.dram_tensor("wqkrs", (P, NQK, KO, P), F8, kind="ExternalInput")
    wvs = nc.dram_tensor("wvs", (P, KO, HPC * HD), F8, kind="ExternalInput")
    wvrs = nc.dram_tensor("wvrs", (P, KO, HPC * HD), F8, kind="ExternalInput")
    wos = nc.dram_tensor("wos", (P, HPC, D), F8, kind="ExternalInput")
    wors = nc.dram_tensor("wors", (P, HPC, D), F8, kind="ExternalInput")
    cs = nc.dram_tensor("cs", (P, 2, T), BF16, kind="ExternalInput")
    masks = nc.dram_tensor("masks", (BANDS, P, TQ), BF16, kind="ExternalInput")
    jT = nc.dram_tensor("jT", (P, P), BF16, kind="ExternalInput")
    ones = nc.dram_tensor("ones", (P, P), BF16, kind="ExternalInput")
    y = nc.dram_tensor("y", (T, D), BF16, kind="ExternalOutput")

    with tile.TileContext(nc) as tc:
        with (
            tc.tile_pool(name="consts", bufs=1) as consts,
            tc.tile_pool(name="qkv", bufs=1) as qkvp,
            tc.tile_pool(name="p2ps", bufs=3, space="PSUM") as p2ps,
        ):
            jT_sb = consts.tile([P, P], BF16)
            mask_sb = consts.tile([P, BANDS, TQ], BF16)
            ones_sb = consts.tile([P, P], BF16)
            cs_sb = consts.tile([P, 2, T], BF16)
            wos_sb = consts.tile([P, HPC, D], F8)
            wors_sb = consts.tile([P, HPC, D], F8)
            const_dmas = [
                lambda: nc.sync.dma_start(cs_sb[:], cs[:]),
                lambda: nc.sync.dma_start(jT_sb[:], jT[:]),
                lambda: nc.sync.dma_start(ones_sb[:], ones[:]),
                lambda: nc.sync.dma_start(mask_sb[:], masks.rearrange("a p j -> p a j")),
                lambda: nc.sync.dma_start(wos_sb[:], wos[:]),
                lambda: nc.sync.dma_start(wors_sb[:], wors[:]),
            ]

            qk_sb = [qkvp.tile([P, T], BF16, name=f"qk{m}") for m in range(NQK)]
            v_sb = [qkvp.tile([P, HPC * HD], BF16, name=f"v{kb}") for kb in range(T // P)]
            qr0 = [qkvp.tile([P, TB], BF16, name=f"qr0_{rb}") for rb in range(NTB)]
            kr0 = [qkvp.tile([P, TB], BF16, name=f"kr0_{rb}") for rb in range(NTB)]

            _phase1(nc, tc, xs, wqks, wqkrs, wvs, wvrs, qk_sb, v_sb,
                    (qr0, kr0, cs_sb, jT_sb), const_dmas)

            with (
                tc.tile_pool(name="outT", bufs=1) as outT_pool,
                tc.tile_pool(name="p3s", bufs=3) as p3s,
                tc.tile_pool(name="p3p", bufs=3, space="PSUM") as p3p,
            ):
                outS = {
                    tq: outT_pool.tile([P, 2 * HPC, TQ], F8, tag=f"outS{tq}", name=f"outS{tq}")
                    for tq in range(NTQ)
                }
                emit_p3 = _make_p3(nc, p3s, p3p, outS, wos_sb, wors_sb, y)
                _phase2(nc, tc, outS, qk_sb, v_sb, jT_sb, mask_sb, ones_sb, cs_sb,
                        (qr0, kr0), emit_p3, p3p, p2ps)

    _fix_waits(nc)
    return nc


_NC_CACHE = None


def _get_program():
    global _NC_CACHE
    if _NC_CACHE is None:
        _NC_CACHE = _build_program()
    return _NC_CACHE


def _q8(a, s):
    """e4m3-quantize a*s (clipped to TRN e4m3 range); returns (fp8, residual
    fp8) with the residual on the same scale (no prescale — its values live
    in e4m3's normal range already)."""
    import ml_dtypes

    F8np = ml_dtypes.float8_e4m3
    scaled = np.clip(a * s, -240.0, 240.0)
    hi = scaled.astype(F8np)
    lo = np.clip(scaled - hi.astype(np.float32), -240.0, 240.0).astype(F8np)
    return hi, lo


def _pack_k(a):
    """[K, M] -> [P, KO', M] with slab i on partitions (rows 128i+p)."""
    ko = a.shape[0] // P
    return np.ascontiguousarray(a.reshape(ko, P, a.shape[1]).transpose(1, 0, 2))


def _host_inputs(x, Wqkv, Wout, cos, sin, rope_mask):
    import ml_dtypes

    BF = ml_dtypes.bfloat16
    x = np.asarray(x, dtype=np.float32)
    Wqkv = np.asarray(Wqkv, dtype=np.float32)
    Wout = np.asarray(Wout, dtype=np.float32)
    cos = np.asarray(cos, dtype=np.float32)
    sin = np.asarray(sin, dtype=np.float32)
    rope_mask = np.asarray(rope_mask).astype(bool)

    # J^T for the pair-rotation matmul: (J q)[2i] = -q[2i+1], (J q)[2i+1] = q[2i]
    jT = np.zeros((P, P), dtype=np.float32)
    for i in range(P // 2):
        jT[2 * i, 2 * i + 1] = 1.0
        jT[2 * i + 1, 2 * i] = -1.0

    masks = np.zeros((BANDS, P, TQ), dtype=BF)
    ii = np.arange(P)[:, None]
    jj = np.arange(TQ)[None, :]
    for a in range(BANDS):
        masks[a] = (ii + a * P <= jj).astype(BF)

    C_full = np.repeat(cos[:T].T, 2, axis=0).astype(np.float32)  # [128, T]
    S_full = np.repeat(sin[:T].T, 2, axis=0).astype(np.float32)

    # per-batch x packs (shared by the 4 cores of each batch)
    xs_b = []
    for b in range(B):
        x8, xr8 = _q8(x[b].T, SX)  # [D, T] fp8
        xsp = np.empty((P, 2 * KO, T), dtype=x8.dtype)
        xsp[:, 0::2] = _pack_k(x8)
        xsp[:, 1::2] = _pack_k(xr8)
        xs_b.append(xsp)

    in_maps = []
    for c in range(N_CORES):
        b = c // CORES_PER_B
        hg = c % CORES_PER_B
        heads = [hg * HPC + i for i in range(HPC)]

        qrows = np.concatenate([np.arange(h * HD, (h + 1) * HD) for h in heads])
        krows = qrows + D
        vrows = qrows + 2 * D
        wqk = Wqkv[np.concatenate([qrows, krows])].T  # [D, 1024]
        wv = Wqkv[vrows].T                            # [D, 512]
        wqk8, wqkr8 = _q8(wqk, SW)
        wv8, wvr8 = _q8(wv, SW)

        def pack_q(a):  # [P, KO, 1024] -> [P, NQK, KO, 128] m-major
            pk = _pack_k(a)
            return np.ascontiguousarray(
                pk.reshape(P, KO, NQK, P).transpose(0, 2, 1, 3)
            )

        woT = np.ascontiguousarray(Wout[:, qrows].T)  # [512, D]
        wo8, wor8 = _q8(woT, SWO)
        wos_p = np.ascontiguousarray(wo8.reshape(HPC, P, D).transpose(1, 0, 2))
        wors_p = np.ascontiguousarray(wor8.reshape(HPC, P, D).transpose(1, 0, 2))

        flags = [bool(rope_mask[h]) for h in heads]
        assert all(f == flags[0] for f in flags), (
            "heads in one core must share a rope flag for the single-table path"
        )
        cs_arr = np.empty((P, 2, T), dtype=BF)
        if flags[0]:
            cs_arr[:, 0] = C_full.astype(BF)
            cs_arr[:, 1] = S_full.astype(BF)
        else:
            cs_arr[:, 0] = np.ones((P, T), dtype=BF)
            cs_arr[:, 1] = np.zeros((P, T), dtype=BF)

        in_maps.append(
            {
                "xs": xs_b[b],
                "wqks": pack_q(wqk8),
                "wqkrs": pack_q(wqkr8),
                "wvs": _pack_k(wv8),
                "wvrs": _pack_k(wvr8),
                "wos": wos_p,
                "wors": wors_p,
                "cs": cs_arr,
                "masks": masks,
                "jT": jT.astype(BF),
                "ones": np.full((P, P), SIGMA / SO, dtype=BF),
            }
        )
    return in_maps


def kernel(x, Wqkv, Wout, cos, sin, rope_mask, _trace=False):
    nc = _get_program()
    in_maps = _host_inputs(x, Wqkv, Wout, cos, sin, rope_mask)
    res = run_bass_kernel_spmd(nc, in_maps, core_ids=list(range(N_CORES)), trace=_trace)
    parts = [np.asarray(res.results[c]["y"], dtype=np.float32) for c in range(N_CORES)]
    out = np.stack(
        [sum(parts[b * CORES_PER_B : (b + 1) * CORES_PER_B]) for b in range(B)]
    ).astype(np.float32)
    if _trace:
        kernel.last_result = res
    return out
